# revision 8
# baseline (speedup 1.0000x reference)
"""ColumnParallelLinear kernel for Trainium2 (8 NeuronCores).

Computes Y[s,b,o] = sum_h X[s,b,h] * W[o,h]  (F.linear / einsum 'sbh,oh->sbo')
with S,B,H,OUT = 2048,4,1024,4096, fp32 in/out.

Strategy:
  - Flatten tokens: M = S*B = 8192 rows.  GEMM: [M,H] @ [H,OUT].
  - 2D shard over 8 cores: 4 token groups (2048 rows) x 2 out-column
    groups (2048 cols) -- minimizes per-core HBM traffic.
  - Inputs cast to bf16 on host (rel err ~3e-3, full-rate PE path).
  - DMA queues are packet-rate limited (~25-35ns per <=8KB packet, one
    packet per partition), so loads are split k-halves spread over the
    four idle engine queues (w: sync+scalar, x: vector+gpsimd) so the
    first tiles land ~3us after queue start.
  - Warmup matmuls on a memset tile keep the PE busy (and its clock
    ramped to 2.4GHz) while the first real tiles arrive.
  - Matmuls: 128x128 bf16 stationary (x), 512-wide moving (w),
    accumulating fp32 in PSUM over 8 k-subtiles; PSUM -> SBUF stage
    rows via vector copies; full 8KB-run row writes on the gpsimd
    queue.  The final row's writes are split by partition range across
    all four queues so the tail drain is ~1us instead of ~4us.
"""

import numpy as np
import ml_dtypes

import concourse.bass as bass
from concourse import bacc
import concourse.mybir as mybir
import concourse.tile as tile
from concourse.bass_utils import run_bass_kernel_spmd

S, B, H, OUT = 2048, 4, 1024, 4096
M = S * B

N_CORES = 8
G_ROW, G_COL = 4, 2          # token groups x out-feature groups
M_LOC = M // G_ROW           # 2048 rows per core
N_LOC = OUT // G_COL         # 2048 out features per core

P = 128
KO = H // P                  # 8 contraction subtiles
KH = KO // 2                 # k-half
NT = 512                     # psum free dim (one fp32 bank)
NO = N_LOC // NT             # 4 col tiles
XG = 512                     # x chunk width (4 row tiles)
NXG = M_LOC // XG            # 4 chunks
MO = M_LOC // P              # 16 row tiles

MM_DT = mybir.dt.bfloat16
N_WARM = 32                  # warmup matmuls during initial DMA wait


def build_nc(mm_dt=MM_DT):
    nc = bacc.Bacc(None, target_bir_lowering=False, enable_partition_id=False)
    # packed inputs: [chunk][partition p][k][free] so each partition's slice
    # of one chunk is 8KB contiguous in DRAM
    xH = nc.declare_dram_parameter("xH", [NXG, P, KO, XG], mm_dt,
                                   isOutput=False)
    wH = nc.declare_dram_parameter("wH", [NO, P, KO, NT], mm_dt,
                                   isOutput=False)
    y = nc.declare_dram_parameter("y", [M_LOC, N_LOC], mybir.dt.float32,
                                  isOutput=True)
    y_r = y[:, :].rearrange("(mo p) n -> p mo n", p=P)

    with tile.TileContext(nc) as tc:
        with (
            tc.tile_pool(name="xp", bufs=1) as xp,
            tc.tile_pool(name="wp", bufs=1) as wp,
            tc.tile_pool(name="op", bufs=4) as op,
            tc.tile_pool(name="psp", bufs=7, space="PSUM") as psp,
            tc.tile_pool(name="psw", bufs=1, space="PSUM") as psw,
        ):
            # ---- PE warmup: matmuls on a zeroed tile, no DMA deps ----
            warm = xp.tile([P, 128 + NT], mm_dt, tag="warm", name="warm")
            nc.vector.memset(warm[:], 0.0)
            wps = psw.tile([P, NT], mybir.dt.float32, tag="warmps",
                           name="warmps")
            for i in range(N_WARM):
                nc.tensor.matmul(wps[:], lhsT=warm[:, :128],
                                 rhs=warm[:, 128:128 + NT],
                                 start=True, stop=True)

            # ---- input loads: k-halves across four engine queues ----
            x_sb = [None] * NXG
            w_sb = [None] * NO
            for g in range(NXG):
                x_sb[g] = xp.tile([P, KO, XG], mm_dt, tag=f"x{g}",
                                  name=f"x{g}")
            for n in range(NO):
                w_sb[n] = wp.tile([P, KO, NT], mm_dt, tag=f"w{n}",
                                  name=f"w{n}")
            for i in range(NXG):          # chunk index in arrival order
                nc.sync.dma_start(w_sb[i][:, 0:KH, :], wH[i, :, 0:KH, :])
                nc.scalar.dma_start(w_sb[i][:, KH:KO, :], wH[i, :, KH:KO, :])
                nc.gpsimd.dma_start(x_sb[i][:], xH[i, :, :, :])

            QUEUES = [nc.sync, nc.scalar, nc.gpsimd]

            def do_group(g, n_outer, tail=False):
                stages = [op.tile([P, N_LOC], mybir.dt.float32, tag=f"st{mi}",
                                  name=f"st{g}_{mi}")
                          for mi in range(XG // P)]
                outer = range(NO) if n_outer else range(XG // P)
                inner = range(XG // P) if n_outer else range(NO)
                for a in outer:
                    for b in inner:
                        n, mi = (a, b) if n_outer else (b, a)
                        ps = psp.tile([P, NT], mybir.dt.float32)
                        for k in range(KO):
                            nc.tensor.matmul(
                                ps[:],
                                lhsT=x_sb[g][:, k, mi * P:(mi + 1) * P],
                                rhs=w_sb[n][:, k, 0:NT],
                                start=(k == 0),
                                stop=(k == KO - 1),
                            )
                        last_block = tail and mi == XG // P - 1 and n == NO - 1
                        if last_block:
                            # split the final copy across two engines and the
                            # final write across all four queues (by partition
                            # range) to shorten the drain after the last matmul
                            HP = P // 2
                            nc.vector.tensor_copy(
                                stages[mi][0:HP, n * NT:(n + 1) * NT],
                                ps[0:HP, :])
                            nc.scalar.copy(
                                stages[mi][HP:P, n * NT:(n + 1) * NT],
                                ps[HP:P, :])
                            mo = g * (XG // P) + mi
                            bounds = [(0, 48), (48, 96), (96, 128)]
                            for (lo, hi), q in zip(bounds, QUEUES):
                                q.dma_start(
                                    y_r[lo:hi, mo, n * NT:(n + 1) * NT],
                                    stages[mi][lo:hi, n * NT:(n + 1) * NT],
                                )
                            continue
                        nc.vector.tensor_copy(
                            stages[mi][:, n * NT:(n + 1) * NT], ps[:]
                        )
                        if tail and mi == XG // P - 1:
                            # per-n write right after each copy, spread over
                            # the queues so no single queue backlogs
                            mo = g * (XG // P) + mi
                            QUEUES[n % 3].dma_start(
                                y_r[:, mo, n * NT:(n + 1) * NT],
                                stages[mi][:, n * NT:(n + 1) * NT],
                            )
                # full 8KB-run row writes on the gpsimd HWDGE ring
                last = XG // P - (1 if tail else 0)
                for mi in range(last):
                    mo = g * (XG // P) + mi
                    nc.gpsimd.dma_start(y_r[:, mo, :], stages[mi][:])

            do_group(0, n_outer=True)     # w arrives n-by-n
            for g in range(1, NXG):
                # mi-outer spreads the writes
                do_group(g, n_outer=False, tail=(g == NXG - 1))

    nc.compile()
    return nc


def make_in_maps(input_, weight):
    X = np.asarray(input_, dtype=np.float32).reshape(M, H)
    W = np.asarray(weight, dtype=np.float32)
    in_maps = []
    for c in range(N_CORES):
        i, j = divmod(c, G_COL)
        # xH[g, p, k, mg] = X[i*M_LOC + g*XG + mg, k*P + p]
        xc = X[i * M_LOC:(i + 1) * M_LOC]                  # [M_LOC, H]
        xh = np.ascontiguousarray(
            xc.reshape(NXG, XG, KO, P).transpose(0, 3, 2, 1)
        ).astype(ml_dtypes.bfloat16)
        # wH[n, p, k, nq] = W[j*N_LOC + n*NT + nq, k*P + p]
        wc = W[j * N_LOC:(j + 1) * N_LOC]                  # [N_LOC, H]
        wh = np.ascontiguousarray(
            wc.reshape(NO, NT, KO, P).transpose(0, 3, 2, 1)
        ).astype(ml_dtypes.bfloat16)
        in_maps.append({"xH": xh, "wH": wh})
    return in_maps


def assemble(results):
    Y = np.empty((M, OUT), dtype=np.float32)
    for c in range(N_CORES):
        i, j = divmod(c, G_COL)
        Y[i * M_LOC:(i + 1) * M_LOC, j * N_LOC:(j + 1) * N_LOC] = results[c]["y"]
    return Y.reshape(S, B, OUT)


def kernel(input_, weight):
    nc = build_nc()
    res = run_bass_kernel_spmd(nc, make_in_maps(input_, weight), list(range(N_CORES)))
    return assemble(res.results)


# revision 11
# speedup vs baseline: 1.0393x; 1.0393x over previous
"""ColumnParallelLinear kernel for Trainium2 (8 NeuronCores).

Computes Y[s,b,o] = sum_h X[s,b,h] * W[o,h]  (F.linear / einsum 'sbh,oh->sbo')
with S,B,H,OUT = 2048,4,1024,4096, fp32 in/out.

Strategy:
  - Flatten tokens: M = S*B = 8192 rows.  GEMM: [M,H] @ [H,OUT].
  - 2D shard over 8 cores: 4 token groups (2048 rows) x 2 out-column
    groups (2048 cols) -- minimizes per-core HBM traffic.
  - Inputs cast to bf16 on host (rel err ~3e-3, full-rate PE path).
  - DMA queues are packet-rate limited (~25-35ns per <=8KB packet, one
    packet per partition), so loads are split k-halves spread over the
    four idle engine queues (w: sync+scalar, x: vector+gpsimd) so the
    first tiles land ~3us after queue start.
  - Warmup matmuls on a memset tile keep the PE busy (and its clock
    ramped to 2.4GHz) while the first real tiles arrive.
  - Matmuls: 128x128 bf16 stationary (x), 512-wide moving (w),
    accumulating fp32 in PSUM over 8 k-subtiles; PSUM -> SBUF stage
    rows via vector copies; full 8KB-run row writes on the gpsimd
    queue.  The final row's writes are split by partition range across
    all four queues so the tail drain is ~1us instead of ~4us.
"""

import numpy as np
import ml_dtypes

import concourse.bass as bass
from concourse import bacc
import concourse.mybir as mybir
import concourse.tile as tile
from concourse.bass_utils import run_bass_kernel_spmd

S, B, H, OUT = 2048, 4, 1024, 4096
M = S * B

N_CORES = 8
G_ROW, G_COL = 4, 2          # token groups x out-feature groups
M_LOC = M // G_ROW           # 2048 rows per core
N_LOC = OUT // G_COL         # 2048 out features per core

P = 128
KO = H // P                  # 8 contraction subtiles
KH = KO // 2                 # k-half
NT = 512                     # psum free dim (one fp32 bank)
NO = N_LOC // NT             # 4 col tiles
XG = 512                     # x chunk width (4 row tiles)
NXG = M_LOC // XG            # 4 chunks
MO = M_LOC // P              # 16 row tiles

MM_DT = mybir.dt.bfloat16
N_WARM = 24                  # warmup matmuls during initial DMA wait


def build_nc(mm_dt=MM_DT):
    nc = bacc.Bacc(None, target_bir_lowering=False, enable_partition_id=False)
    # packed inputs: [chunk][partition p][k][free] so each partition's slice
    # of one chunk is 8KB contiguous in DRAM
    xH = nc.declare_dram_parameter("xH", [NXG, P, KO, XG], mm_dt,
                                   isOutput=False)
    wH = nc.declare_dram_parameter("wH", [NO, P, KO, NT], mm_dt,
                                   isOutput=False)
    y = nc.declare_dram_parameter("y", [M_LOC, N_LOC], mybir.dt.float32,
                                  isOutput=True)
    y_r = y[:, :].rearrange("(mo p) n -> p mo n", p=P)

    with tile.TileContext(nc) as tc:
        with (
            tc.tile_pool(name="xp", bufs=1) as xp,
            tc.tile_pool(name="wp", bufs=1) as wp,
            tc.tile_pool(name="op", bufs=4) as op,
            tc.tile_pool(name="psp", bufs=7, space="PSUM") as psp,
            tc.tile_pool(name="psw", bufs=1, space="PSUM") as psw,
        ):
            # ---- PE warmup: matmuls on a zeroed tile, no DMA deps ----
            warm = xp.tile([P, 128 + NT], mm_dt, tag="warm", name="warm")
            nc.vector.memset(warm[:], 0.0)
            wps = psw.tile([P, NT], mybir.dt.float32, tag="warmps",
                           name="warmps")
            for i in range(N_WARM):
                nc.tensor.matmul(wps[:], lhsT=warm[:, :128],
                                 rhs=warm[:, 128:128 + NT],
                                 start=True, stop=True)

            # ---- input loads: k-halves across four engine queues ----
            x_sb = [None] * NXG
            w_sb = [None] * NO
            for g in range(NXG):
                x_sb[g] = xp.tile([P, KO, XG], mm_dt, tag=f"x{g}",
                                  name=f"x{g}")
            for n in range(NO):
                w_sb[n] = wp.tile([P, KO, NT], mm_dt, tag=f"w{n}",
                                  name=f"w{n}")
            # whole-chunk loads only (each dma_start costs ~128 packets
            # regardless of size); sync queue starts earliest -> w0 first,
            # scalar gets x0; gpsimd (latest/slowest queue) only writes y
            nc.sync.dma_start(w_sb[0][:], wH[0, :, :, :])
            nc.scalar.dma_start(x_sb[0][:], xH[0, :, :, :])
            nc.sync.dma_start(x_sb[1][:], xH[1, :, :, :])
            nc.scalar.dma_start(w_sb[1][:], wH[1, :, :, :])
            nc.sync.dma_start(w_sb[2][:], wH[2, :, :, :])
            nc.scalar.dma_start(x_sb[2][:], xH[2, :, :, :])
            nc.sync.dma_start(x_sb[3][:], xH[3, :, :, :])
            nc.scalar.dma_start(w_sb[3][:], wH[3, :, :, :])

            QUEUES = [nc.sync, nc.scalar, nc.gpsimd]
            rr = [0]  # round-robin cursor for y writes

            def write_row(mo, stage):
                q = QUEUES[rr[0] % 3]
                rr[0] += 1
                q.dma_start(y_r[:, mo, :], stage[:])

            def do_group(g, n_outer, tail=False):
                stages = [op.tile([P, N_LOC], mybir.dt.float32, tag=f"st{mi}",
                                  name=f"st{g}_{mi}")
                          for mi in range(XG // P)]
                outer = range(NO) if n_outer else range(XG // P)
                inner = range(XG // P) if n_outer else range(NO)
                for a in outer:
                    for b in inner:
                        n, mi = (a, b) if n_outer else (b, a)
                        ps = psp.tile([P, NT], mybir.dt.float32)
                        for k in range(KO):
                            nc.tensor.matmul(
                                ps[:],
                                lhsT=x_sb[g][:, k, mi * P:(mi + 1) * P],
                                rhs=w_sb[n][:, k, 0:NT],
                                start=(k == 0),
                                stop=(k == KO - 1),
                            )
                        last_block = tail and mi == XG // P - 1 and n == NO - 1
                        if last_block:
                            # split the final copy across two engines and the
                            # final write across all four queues (by partition
                            # range) to shorten the drain after the last matmul
                            HP = P // 2
                            nc.vector.tensor_copy(
                                stages[mi][0:HP, n * NT:(n + 1) * NT],
                                ps[0:HP, :])
                            nc.scalar.copy(
                                stages[mi][HP:P, n * NT:(n + 1) * NT],
                                ps[HP:P, :])
                            mo = g * (XG // P) + mi
                            bounds = [(0, 48), (48, 96), (96, 128)]
                            for (lo, hi), q in zip(bounds, QUEUES):
                                q.dma_start(
                                    y_r[lo:hi, mo, n * NT:(n + 1) * NT],
                                    stages[mi][lo:hi, n * NT:(n + 1) * NT],
                                )
                            continue
                        nc.vector.tensor_copy(
                            stages[mi][:, n * NT:(n + 1) * NT], ps[:]
                        )
                        if tail and mi == XG // P - 1:
                            # per-n write right after each copy, partition-
                            # split across all three queues so no single
                            # 128-packet write gates the tail
                            mo = g * (XG // P) + mi
                            bounds = [(0, 48), (48, 96), (96, 128)]
                            for (lo, hi), q in zip(bounds, QUEUES):
                                q.dma_start(
                                    y_r[lo:hi, mo, n * NT:(n + 1) * NT],
                                    stages[mi][lo:hi, n * NT:(n + 1) * NT],
                                )
                # full 8KB-run row writes, round-robin over the queues
                last = XG // P - (1 if tail else 0)
                for mi in range(last):
                    mo = g * (XG // P) + mi
                    write_row(mo, stages[mi])

            do_group(0, n_outer=True)     # w arrives n-by-n
            for g in range(1, NXG):
                # mi-outer spreads the writes
                do_group(g, n_outer=False, tail=(g == NXG - 1))

    nc.compile()
    return nc


def make_in_maps(input_, weight):
    X = np.asarray(input_, dtype=np.float32).reshape(M, H)
    W = np.asarray(weight, dtype=np.float32)
    in_maps = []
    for c in range(N_CORES):
        i, j = divmod(c, G_COL)
        # xH[g, p, k, mg] = X[i*M_LOC + g*XG + mg, k*P + p]
        xc = X[i * M_LOC:(i + 1) * M_LOC]                  # [M_LOC, H]
        xh = np.ascontiguousarray(
            xc.reshape(NXG, XG, KO, P).transpose(0, 3, 2, 1)
        ).astype(ml_dtypes.bfloat16)
        # wH[n, p, k, nq] = W[j*N_LOC + n*NT + nq, k*P + p]
        wc = W[j * N_LOC:(j + 1) * N_LOC]                  # [N_LOC, H]
        wh = np.ascontiguousarray(
            wc.reshape(NO, NT, KO, P).transpose(0, 3, 2, 1)
        ).astype(ml_dtypes.bfloat16)
        in_maps.append({"xH": xh, "wH": wh})
    return in_maps


def assemble(results):
    Y = np.empty((M, OUT), dtype=np.float32)
    for c in range(N_CORES):
        i, j = divmod(c, G_COL)
        Y[i * M_LOC:(i + 1) * M_LOC, j * N_LOC:(j + 1) * N_LOC] = results[c]["y"]
    return Y.reshape(S, B, OUT)


def kernel(input_, weight):
    nc = build_nc()
    res = run_bass_kernel_spmd(nc, make_in_maps(input_, weight), list(range(N_CORES)))
    return assemble(res.results)
